# revision 1
# baseline (speedup 1.0000x reference)
"""DeformConv1d (modulated, K=3, stride=1, pad=1, dil=1) on 8 Trainium2
NeuronCores via Bass.

Contract: kernel(**inputs) takes the FULL inputs
  x[16,256,4096] f32, weight[256,256,3] f32, offset[16,3,4096] f32,
  mask[16,3,4096] f32, bias[256] f32
and returns the full output [16,256,4096] f32.

Strategy (data-parallel over batch, 2 batches per core):
  out[b,o,w] = sum_k m[k,w]*(w0*z_k[o,i0] + w1*z_k[o,i0+1]) + bias[o],
  z_k = W_k @ x[b].
  On device, z_k is produced TRANSPOSED ([w,oc] tiles) by matmuls with
  x-slices stationary, staged to DRAM, then two indirect-DMA row gathers
  per tap fetch z_k[i0]/z_k[i0+1]; VectorE applies the interpolation
  weights (precomputed host-side from offset/mask, along with clamped
  indices) and the bias. The output leaves the device transposed and is
  unpermuted on the host.
"""
import numpy as np

import concourse.bass as bass
import concourse.bacc as bacc
import concourse.tile as tile
from concourse import mybir
from concourse.bass_utils import run_bass_kernel_spmd

F32 = mybir.dt.float32
F32R = mybir.dt.float32r
BF16 = mybir.dt.bfloat16
I32 = mybir.dt.int32

B2 = 2          # batches per core
K = 3
W = 4096
NT = W // 128   # 32 w-tiles
N_CORES = 8


def _build(reps: int = 1, fast: bool = False, t1_dve: bool = False, deep: bool = False):
    nc = bacc.Bacc("TRN2", target_bir_lowering=False, debug=False)

    z_dt = F32  # keep staging/gather fp32 for precision; fast only switches matmul to fp32r

    x_in = nc.dram_tensor("x_in", [B2, 128, 2, W], F32, kind="ExternalInput")
    wT_in = nc.dram_tensor("wT_in", [128, K, 2, 256], F32, kind="ExternalInput")
    bias_in = nc.dram_tensor("bias_in", [128, 256], F32, kind="ExternalInput")
    idx0_in = nc.dram_tensor("idx0_in", [B2, 128, K, NT], I32, kind="ExternalInput")
    idx1_in = nc.dram_tensor("idx1_in", [B2, 128, K, NT], I32, kind="ExternalInput")
    c0_in = nc.dram_tensor("c0_in", [B2, 128, K, NT], F32, kind="ExternalInput")
    c1_in = nc.dram_tensor("c1_in", [B2, 128, K, NT], F32, kind="ExternalInput")
    outT = nc.dram_tensor("outT", [B2, 128, NT, 256], F32, kind="ExternalOutput")
    zTs = [[nc.dram_tensor(f"zT_{b}_{k}", [W, 256], z_dt) for k in range(K)]
           for b in range(B2)]

    with tile.TileContext(nc) as tc:
        with (
            tc.tile_pool(name="const", bufs=1) as cpool,
            tc.tile_pool(name="xp", bufs=1) as xpool,
            tc.tile_pool(name="zstage", bufs=8 if deep else 6) as zpool,
            tc.tile_pool(name="gp", bufs=1) as gpool,
            tc.tile_pool(name="coefp", bufs=2) as coefpool,
            tc.tile_pool(name="interp", bufs=2) as ipool,
            tc.tile_pool(name="accp", bufs=1) as apool,
            tc.tile_pool(name="psum", bufs=8, space="PSUM") as psum,
        ):
            w_raw = cpool.tile([128, K, 2, 256], F32, tag="wraw")
            nc.sync.dma_start(out=w_raw[:], in_=wT_in[:])
            if fast:
                w_rt = cpool.tile([128, K, 2, 256], F32R, tag="wr")
                nc.vector.tensor_copy(w_rt[:], w_raw[:])
                w_r = w_rt[:]
            else:
                w_r = w_raw[:]
            bias_sb = cpool.tile([128, 256], F32, tag="bias")
            nc.sync.dma_start(out=bias_sb[:], in_=bias_in[:])
            zero_b = cpool.tile([128, 1], F32, tag="zb")
            nc.gpsimd.memset(zero_b[:], 0.0)

            for rep in range(reps):
                for b in range(B2):
                    x_raw = xpool.tile([128, 2, W], F32, tag="xraw")
                    nc.sync.dma_start(out=x_raw[:], in_=x_in[b])
                    if fast:
                        x_rt = xpool.tile([128, 2, W], F32R, tag="xr")
                        nc.vector.tensor_copy(x_rt[:], x_raw[:])
                        x_r = x_rt[:]
                    else:
                        x_r = x_raw[:]

                    idx0_sb = coefpool.tile([128, K, NT], I32, tag="i0")
                    idx1_sb = coefpool.tile([128, K, NT], I32, tag="i1")
                    c0_sb = coefpool.tile([128, K, NT], F32, tag="c0")
                    c1_sb = coefpool.tile([128, K, NT], F32, tag="c1")
                    nc.sync.dma_start(out=idx0_sb[:], in_=idx0_in[b])
                    nc.sync.dma_start(out=idx1_sb[:], in_=idx1_in[b])
                    nc.sync.dma_start(out=c0_sb[:], in_=c0_in[b])
                    nc.sync.dma_start(out=c1_sb[:], in_=c1_in[b])

                    acc = apool.tile([128, NT, 256], F32, tag="acc")

                    for k in range(K):
                        for wt in range(NT):
                            zp = psum.tile([128, 256], F32, tag="zp")
                            ws = wt * 128
                            for cc in range(2):
                                nc.tensor.matmul(
                                    zp[:],
                                    x_r[:, cc, ws:ws + 128],
                                    w_r[:, k, cc],
                                    start=(cc == 0),
                                    stop=(cc == 1),
                                )
                            zst = zpool.tile([128, 256], z_dt, tag="zst")
                            nc.scalar.activation(
                                zst[:], zp[:],
                                mybir.ActivationFunctionType.Identity,
                                bias=zero_b[:])
                            nc.sync.dma_start(
                                out=zTs[b][k][ws:ws + 128], in_=zst[:])

                        zrows = zTs[b][k][:]  # [4096, 256]
                        H = NT // 2
                        for h in range(2):
                            hs = h * H
                            g0 = gpool.tile([128, H, 256], z_dt, tag="g0")
                            g1 = gpool.tile([128, H, 256], z_dt, tag="g1")
                            for t in range(H):
                                nc.gpsimd.indirect_dma_start(
                                    out=g0[:, t], out_offset=None, in_=zrows,
                                    in_offset=bass.IndirectOffsetOnAxis(
                                        ap=idx0_sb[:, k, hs + t:hs + t + 1],
                                        axis=0))
                                nc.gpsimd.indirect_dma_start(
                                    out=g1[:, t], out_offset=None, in_=zrows,
                                    in_offset=bass.IndirectOffsetOnAxis(
                                        ap=idx1_sb[:, k, hs + t:hs + t + 1],
                                        axis=0))

                            c0b = c0_sb[:, k, hs:hs + H][:, :, None] \
                                .broadcast_to([128, H, 256])
                            c1b = c1_sb[:, k, hs:hs + H][:, :, None] \
                                .broadcast_to([128, H, 256])
                            t0 = ipool.tile([128, H, 256], F32, tag="t0")
                            acch = acc[:, hs:hs + H]
                            nc.vector.tensor_tensor(t0[:], g0[:], c0b,
                                                    mybir.AluOpType.mult)
                            if k == 0:
                                biasb = bias_sb[:][:, None, :].broadcast_to(
                                    [128, H, 256])
                                nc.vector.tensor_tensor(
                                    acch, t0[:], biasb, mybir.AluOpType.add)
                            else:
                                nc.vector.tensor_tensor(
                                    acch, acch, t0[:], mybir.AluOpType.add)
                            t1 = ipool.tile([128, H, 256], F32, tag="t0")
                            eng_t1 = nc.vector if t1_dve else nc.gpsimd
                            eng_t1.tensor_tensor(t1[:], g1[:], c1b,
                                                 mybir.AluOpType.mult)
                            nc.vector.tensor_tensor(
                                acch, acch, t1[:], mybir.AluOpType.add)

                    nc.sync.dma_start(out=outT[b], in_=acc[:])

    nc.compile()
    return nc


def _prep_coeffs(offset, mask):
    """offset/mask [B,K,W] -> idx0,c0,idx1,c1 in [B,128,K,NT] device layout,
    slot (p,t) <-> w = t*128+p. Float op order replicates the reference."""
    B = offset.shape[0]
    base = np.arange(W, dtype=np.float32) * np.float32(1.0) - np.float32(1.0)
    kpos = np.arange(K, dtype=np.float32) * np.float32(1.0)
    bk = (base[None, :] + kpos[:, None]).astype(np.float32)
    p = (bk[None] + offset).astype(np.float32)
    i0f = np.floor(p)
    w1 = (p - i0f).astype(np.float32)
    w0 = (np.float32(1.0) - w1).astype(np.float32)
    i0 = i0f.astype(np.int64)
    i1 = i0 + 1
    v0 = (i0 >= 0) & (i0 < W)
    v1 = (i1 >= 0) & (i1 < W)
    c0 = (mask * w0 * v0).astype(np.float32)
    c1 = (mask * w1 * v1).astype(np.float32)
    idx0 = np.clip(i0, 0, W - 1).astype(np.int32)
    idx1 = np.clip(i1, 0, W - 1).astype(np.int32)

    def lay(a):
        return np.ascontiguousarray(a.reshape(B, K, NT, 128).transpose(0, 3, 1, 2))

    return lay(idx0), lay(c0), lay(idx1), lay(c1)


def _core_inputs(x, weight, offset, mask, bias, core):
    b0 = 2 * core
    idx0, c0, idx1, c1 = _prep_coeffs(offset[b0:b0 + 2], mask[b0:b0 + 2])
    OC = weight.shape[0]
    return {
        "x_in": np.ascontiguousarray(
            x[b0:b0 + 2].reshape(2, 2, 128, W).transpose(0, 2, 1, 3)
        ).astype(np.float32),
        "wT_in": np.ascontiguousarray(
            weight.transpose(2, 1, 0).reshape(K, 2, 128, OC)
            .transpose(2, 0, 1, 3)).astype(np.float32),
        "bias_in": np.ascontiguousarray(
            np.broadcast_to(bias.reshape(1, -1), (128, OC))).astype(np.float32),
        "idx0_in": idx0, "idx1_in": idx1, "c0_in": c0, "c1_in": c1,
    }


_NC_CACHE = {}


def _get_nc(reps=1, fast=False):
    key = (reps, fast)
    if key not in _NC_CACHE:
        _NC_CACHE[key] = _build(reps=reps, fast=fast)
    return _NC_CACHE[key]


_DISPATCH = None


def _get_dispatch(nc):
    """Build (once) a cached jitted shard_map dispatcher over 8 cores,
    mirroring bass2jax.run_bass_via_pjrt but without per-call retracing."""
    global _DISPATCH
    if _DISPATCH is not None:
        return _DISPATCH
    import jax
    from jax.sharding import Mesh, PartitionSpec
    from jax.experimental.shard_map import shard_map
    from concourse import bass2jax, mybir as mb
    bass2jax.install_neuronx_cc_hook()

    partition_name = (nc.partition_id_tensor.name
                      if nc.partition_id_tensor else None)
    in_names, out_names, out_avals, zero_outs = [], [], [], []
    for alloc in nc.m.functions[0].allocations:
        if not isinstance(alloc, mb.MemoryLocationSet):
            continue
        name = alloc.memorylocations[0].name
        if alloc.kind == "ExternalInput":
            if name != partition_name:
                in_names.append(name)
        elif alloc.kind == "ExternalOutput":
            shape = tuple(alloc.tensor_shape)
            dtype = mb.dt.np(alloc.dtype)
            out_names.append(name)
            out_avals.append(jax.core.ShapedArray(shape, dtype))
            zero_outs.append(np.zeros(shape, dtype))
    n_params = len(in_names)
    n_outs = len(out_avals)
    all_in_names = list(in_names) + list(out_names)
    if partition_name is not None:
        all_in_names.append(partition_name)

    def _body(*args):
        operands = list(args)
        if partition_name is not None:
            operands.append(bass2jax.partition_id_tensor())
        outs = bass2jax._bass_exec_p.bind(
            *operands,
            out_avals=tuple(out_avals),
            in_names=tuple(all_in_names),
            out_names=tuple(out_names),
            lowering_input_output_aliases=(),
            sim_require_finite=True,
            sim_require_nnan=True,
            nc=nc,
        )
        return tuple(outs)

    devices = jax.devices()[:N_CORES]
    mesh = Mesh(np.asarray(devices), ("core",))
    in_specs = (PartitionSpec("core"),) * (n_params + n_outs)
    out_specs = (PartitionSpec("core"),) * n_outs
    donate = tuple(range(n_params, n_params + n_outs))
    sharded = jax.jit(
        shard_map(_body, mesh=mesh, in_specs=in_specs, out_specs=out_specs,
                  check_rep=False),
        donate_argnums=donate, keep_unused=True)
    _DISPATCH = (sharded, in_names, out_names, out_avals, zero_outs)
    return _DISPATCH


def kernel(x, weight, offset, mask, bias):
    x = np.asarray(x, dtype=np.float32)
    weight = np.asarray(weight, dtype=np.float32)
    offset = np.asarray(offset, dtype=np.float32)
    mask = np.asarray(mask, dtype=np.float32)
    bias = np.asarray(bias, dtype=np.float32)

    nc = _get_nc(fast=True)
    sharded, in_names, out_names, out_avals, zero_outs = _get_dispatch(nc)
    ins_list = [_core_inputs(x, weight, offset, mask, bias, core)
                for core in range(N_CORES)]
    concat_in = [np.concatenate([ins_list[c][n] for c in range(N_CORES)],
                                axis=0) for n in in_names]
    concat_zeros = [np.zeros((N_CORES * z.shape[0], *z.shape[1:]), z.dtype)
                    for z in zero_outs]
    out_arrs = sharded(*concat_in, *concat_zeros)
    i = out_names.index("outT")
    allT = np.asarray(out_arrs[i]).reshape(N_CORES, *out_avals[i].shape)

    out = np.empty((16, 256, W), np.float32)
    for core in range(N_CORES):
        out[2 * core:2 * core + 2] = np.ascontiguousarray(
            allT[core].transpose(0, 3, 2, 1).reshape(2, 256, W))
    return out



# revision 11
# speedup vs baseline: 1.3953x; 1.3953x over previous
"""DeformConv1d (modulated, K=3, stride=1, pad=1, dil=1) on 8 Trainium2
NeuronCores via Bass.

Contract: kernel(**inputs) takes the FULL inputs
  x[16,256,4096] f32, weight[256,256,3] f32, offset[16,3,4096] f32,
  mask[16,3,4096] f32, bias[256] f32
and returns the full output [16,256,4096] f32.

Strategy (data-parallel over batch, 2 batches per core), all on-chip:
  out[b,o,w] = sum_k sum_j A_k[j,w] * z_k[o,j] + bias[o],   z_k = W_k @ x[b]
  where A_k[j,w] = c0_k[w]*[j==i0(k,w)] + c1_k[w]*[j==i1(k,w)] is a banded
  coefficient matrix built host-side from offset/mask (2 nnz per column,
  |j-w| <= 6). Per 128-wide output tile the band is covered by one dense
  128x128 block plus 16-row halo strips from the neighbor z tiles, so the
  deformable interpolation becomes three small PSUM-accumulated matmuls on
  the tensor engine. No DRAM staging of z, no indirect-DMA gathers, no
  vector-engine interpolation. Bias is folded into the PSUM->SBUF copy on
  the activation engine; the output leaves the device in final [oc,w]
  layout.
"""
import numpy as np
import ml_dtypes

import concourse.bass as bass
import concourse.bacc as bacc
import concourse.tile as tile
from concourse import mybir
from concourse.bass_utils import run_bass_kernel_spmd  # noqa: F401  (env check)

F32 = mybir.dt.float32
F32R = mybir.dt.float32r
BF16 = mybir.dt.bfloat16

B2 = 2          # batches per core
K = 3
W = 4096
NT = W // 128   # 32 w-tiles
HALO = 16       # halo rows on each side of a 128-row z tile
N_CORES = 8
BF = ml_dtypes.bfloat16


def _build(reps: int = 1, fast: bool = True):
    nc = bacc.Bacc("TRN2", target_bir_lowering=False, debug=False)

    x_in = nc.dram_tensor("x_in", [B2, 128, 2, W], F32R, kind="ExternalInput")
    wT_in = nc.dram_tensor("wT_in", [128, K, 2, 256], F32R, kind="ExternalInput")
    bias_in = nc.dram_tensor("bias_in", [128, 2], F32, kind="ExternalInput")
    a0_in = nc.dram_tensor("a0_in", [B2, 128, K, NT, 128], BF16,
                           kind="ExternalInput")
    # left-halo strip operand lives at base partition 64 (rows 48..63 of the
    # uploaded 64-row block are the real d in [-HALO,0) band, rest zero);
    # right-halo at base 0 (rows 0..15 real, 16..31 zero) — matmul operands
    # must start at partition 0/32/64.
    am_in = nc.dram_tensor("am_in", [B2, 64, K, NT, HALO], BF16,
                           kind="ExternalInput")
    ap_in = nc.dram_tensor("ap_in", [B2, 32, K, NT, HALO], BF16,
                           kind="ExternalInput")
    out_d = nc.dram_tensor("out_d", [B2, 2, 128, W], F32, kind="ExternalOutput")

    with tile.TileContext(nc) as tc:
        with (
            tc.tile_pool(name="const", bufs=1) as cpool,
            tc.tile_pool(name="xp", bufs=2) as xpool,
            tc.tile_pool(name="ap", bufs=2) as apool,
            tc.tile_pool(name="zsb", bufs=5) as zpool,
            tc.tile_pool(name="osb", bufs=2) as opool,
            tc.tile_pool(name="psz", bufs=3, space="PSUM") as psz,
            tc.tile_pool(name="psc", bufs=2, space="PSUM") as psc,
        ):
            w_sb = cpool.tile([128, K, 2, 256], F32R, tag="w")
            nc.sync.dma_start(out=w_sb[:], in_=wT_in[:])
            bias_sb = cpool.tile([128, 2], F32, tag="bias")
            nc.sync.dma_start(out=bias_sb[:], in_=bias_in[:])
            zero_b = cpool.tile([128, 1], F32, tag="zb")
            nc.gpsimd.memset(zero_b[:], 0.0)

            for rep in range(reps):
                for b in range(B2):
                    x_r = xpool.tile([128, 2, W], F32R, tag="x")
                    nc.sync.dma_start(out=x_r[:], in_=x_in[b])
                    a0_sb = apool.tile([128, K, NT, 128], BF16, tag="a0")
                    nc.sync.dma_start(out=a0_sb[:], in_=a0_in[b])
                    am_sb = apool.tile([128, K, NT, HALO], BF16, tag="am")
                    nc.sync.dma_start(out=am_sb[64:128], in_=am_in[b])
                    ap_sb = apool.tile([128, K, NT, HALO], BF16, tag="apv")
                    nc.sync.dma_start(out=ap_sb[0:32], in_=ap_in[b])

                    ost = [opool.tile([128, 1024], F32, tag=f"os{occ}",
                                      name=f"ost{occ}")
                           for occ in range(2)]
                    zt = [[None] * NT for _ in range(K)]

                    def combine(tc_):
                        nonlocal ost
                        for occ in range(2):
                            P = psc.tile([128, 128], F32, tag=f"P{occ}")
                            mms = []
                            for k in range(K):
                                mms.append((P[:, :],
                                            zt[k][tc_][:, occ * 128:occ * 128 + 128],
                                            a0_sb[:, k, tc_]))
                                if tc_ > 0:
                                    mms.append((P[:, 0:HALO],
                                                zt[k][tc_ - 1][64:128,
                                                               occ * 128:occ * 128 + 128],
                                                am_sb[64:128, k, tc_]))
                                if tc_ < NT - 1:
                                    mms.append((P[:, 128 - HALO:128],
                                                zt[k][tc_ + 1][0:32,
                                                               occ * 128:occ * 128 + 128],
                                                ap_sb[0:32, k, tc_]))
                            for i, (o, l, r) in enumerate(mms):
                                nc.tensor.matmul(o, l, r, start=(i == 0),
                                                 stop=(i == len(mms) - 1))
                            nc.scalar.activation(
                                ost[occ][:, (tc_ % 8) * 128:(tc_ % 8) * 128 + 128],
                                P[:],
                                mybir.ActivationFunctionType.Identity,
                                bias=bias_sb[:, occ:occ + 1])
                        if tc_ % 8 == 7:
                            g = tc_ // 8
                            for occ in range(2):
                                nc.gpsimd.dma_start(
                                    out=out_d[b, occ, :, g * 1024:g * 1024 + 1024],
                                    in_=ost[occ][:])
                            if tc_ < NT - 1:
                                ost = [opool.tile([128, 1024], F32,
                                                  tag=f"os{occ}",
                                                  name=f"ost{occ}")
                                       for occ in range(2)]

                    for t in range(NT):
                        ws = t * 128
                        for k in range(K):
                            zp = psz.tile([128, 256], F32, tag="zp")
                            for cc in range(2):
                                nc.tensor.matmul(
                                    zp[:],
                                    x_r[:, cc, ws:ws + 128],
                                    w_sb[:, k, cc],
                                    start=(cc == 0),
                                    stop=(cc == 1),
                                )
                            zs = zpool.tile([128, 256], BF16, tag=f"z{k}")
                            zt[k][t] = zs
                            if (t * K + k) % 5 < 3:
                                nc.vector.tensor_copy(zs[:], zp[:])
                            else:
                                nc.scalar.activation(
                                    zs[:], zp[:],
                                    mybir.ActivationFunctionType.Identity,
                                    bias=zero_b[:])
                        if t >= 1:
                            combine(t - 1)
                    combine(NT - 1)

    nc.compile()
    return nc


def _prep_A(offset, mask):
    """offset/mask [B,K,W] -> banded coefficient blocks for the combine
    matmuls, in device layout (partition dim first):
      a0 [B,128,K,NT,128], am/ap [B,HALO,K,NT,HALO] (bf16).
    Float op order replicates the reference."""
    B = offset.shape[0]
    base = np.arange(W, dtype=np.float32) - np.float32(1.0)
    kpos = np.arange(K, dtype=np.float32)
    bk = (base[None, :] + kpos[:, None]).astype(np.float32)
    p = (bk[None] + offset).astype(np.float32)
    i0f = np.floor(p)
    w1 = (p - i0f).astype(np.float32)
    w0 = (np.float32(1.0) - w1).astype(np.float32)
    i0 = i0f.astype(np.int64)
    i1 = i0 + 1
    c0 = (mask * w0 * ((i0 >= 0) & (i0 < W))).astype(np.float32)
    c1 = (mask * w1 * ((i1 >= 0) & (i1 < W))).astype(np.float32)

    A0 = np.zeros((B, K, NT, 128, 128), np.float32)
    Am = np.zeros((B, K, NT, 64, HALO), np.float32)
    Ap = np.zeros((B, K, NT, 32, HALO), np.float32)
    bb, kk, ww = np.meshgrid(np.arange(B), np.arange(K), np.arange(W),
                             indexing="ij")
    ttt = ww // 128
    wl = ww % 128
    for i, c in ((i0, c0), (i1, c1)):
        d = i - ttt * 128
        nz = c != 0.0
        m0 = nz & (d >= 0) & (d < 128)
        np.add.at(A0, (bb[m0], kk[m0], ttt[m0], d[m0], wl[m0]), c[m0])
        mm_ = nz & (d >= -HALO) & (d < 0)
        np.add.at(Am, (bb[mm_], kk[mm_], ttt[mm_], d[mm_] + 64,
                       wl[mm_]), c[mm_])
        mp = nz & (d >= 128) & (d < 128 + HALO)
        np.add.at(Ap, (bb[mp], kk[mp], ttt[mp], d[mp] - 128,
                       wl[mp] - (128 - HALO)), c[mp])
        assert not (nz & ((d < -HALO) | (d >= 128 + HALO))).any(), \
            "deform offset exceeds halo width"

    def lay(a):
        return np.ascontiguousarray(a.transpose(0, 3, 1, 2, 4)).astype(BF)

    return lay(A0), lay(Am), lay(Ap)


def _core_inputs(x, weight, offset, mask, bias, core):
    b0 = 2 * core
    a0, am, ap = _prep_A(offset[b0:b0 + 2], mask[b0:b0 + 2])
    OC = weight.shape[0]
    return {
        "x_in": np.ascontiguousarray(
            x[b0:b0 + 2].reshape(2, 2, 128, W).transpose(0, 2, 1, 3)
        ).astype(np.float32),
        "wT_in": np.ascontiguousarray(
            weight.transpose(2, 1, 0).reshape(K, 2, 128, OC)
            .transpose(2, 0, 1, 3)).astype(np.float32),
        "bias_in": np.ascontiguousarray(
            bias.reshape(2, 128).T).astype(np.float32),
        "a0_in": a0, "am_in": am, "ap_in": ap,
    }


_NC_CACHE = {}


def _get_nc(reps=1, fast=True):
    key = (reps,)
    if key not in _NC_CACHE:
        _NC_CACHE[key] = _build(reps=reps)
    return _NC_CACHE[key]


_DISPATCH = None


def _get_dispatch(nc):
    """Build (once) a cached jitted shard_map dispatcher over 8 cores,
    mirroring bass2jax.run_bass_via_pjrt but without per-call retracing."""
    global _DISPATCH
    if _DISPATCH is not None:
        return _DISPATCH
    import jax
    from jax.sharding import Mesh, PartitionSpec
    from jax.experimental.shard_map import shard_map
    from concourse import bass2jax, mybir as mb
    bass2jax.install_neuronx_cc_hook()

    partition_name = (nc.partition_id_tensor.name
                      if nc.partition_id_tensor else None)
    in_names, out_names, out_avals, zero_outs = [], [], [], []
    for alloc in nc.m.functions[0].allocations:
        if not isinstance(alloc, mb.MemoryLocationSet):
            continue
        name = alloc.memorylocations[0].name
        if alloc.kind == "ExternalInput":
            if name != partition_name:
                in_names.append(name)
        elif alloc.kind == "ExternalOutput":
            shape = tuple(alloc.tensor_shape)
            dtype = mb.dt.np(alloc.dtype)
            out_names.append(name)
            out_avals.append(jax.core.ShapedArray(shape, dtype))
            zero_outs.append(np.zeros(shape, dtype))
    n_params = len(in_names)
    n_outs = len(out_avals)
    all_in_names = list(in_names) + list(out_names)
    if partition_name is not None:
        all_in_names.append(partition_name)

    def _body(*args):
        operands = list(args)
        if partition_name is not None:
            operands.append(bass2jax.partition_id_tensor())
        outs = bass2jax._bass_exec_p.bind(
            *operands,
            out_avals=tuple(out_avals),
            in_names=tuple(all_in_names),
            out_names=tuple(out_names),
            lowering_input_output_aliases=(),
            sim_require_finite=True,
            sim_require_nnan=True,
            nc=nc,
        )
        return tuple(outs)

    devices = jax.devices()[:N_CORES]
    mesh = Mesh(np.asarray(devices), ("core",))
    in_specs = (PartitionSpec("core"),) * (n_params + n_outs)
    out_specs = (PartitionSpec("core"),) * n_outs
    donate = tuple(range(n_params, n_params + n_outs))
    sharded = jax.jit(
        shard_map(_body, mesh=mesh, in_specs=in_specs, out_specs=out_specs,
                  check_rep=False),
        donate_argnums=donate, keep_unused=True)
    _DISPATCH = (sharded, in_names, out_names, out_avals, zero_outs)
    return _DISPATCH


def kernel(x, weight, offset, mask, bias):
    x = np.asarray(x, dtype=np.float32)
    weight = np.asarray(weight, dtype=np.float32)
    offset = np.asarray(offset, dtype=np.float32)
    mask = np.asarray(mask, dtype=np.float32)
    bias = np.asarray(bias, dtype=np.float32)

    nc = _get_nc()
    sharded, in_names, out_names, out_avals, zero_outs = _get_dispatch(nc)
    ins_list = [_core_inputs(x, weight, offset, mask, bias, core)
                for core in range(N_CORES)]
    concat_in = [np.concatenate([ins_list[c][n] for c in range(N_CORES)],
                                axis=0) for n in in_names]
    concat_zeros = [np.zeros((N_CORES * z.shape[0], *z.shape[1:]), z.dtype)
                    for z in zero_outs]
    out_arrs = sharded(*concat_in, *concat_zeros)
    i = out_names.index("out_d")
    allT = np.asarray(out_arrs[i]).reshape(N_CORES, *out_avals[i].shape)
    return np.ascontiguousarray(allT.reshape(16, 256, W))
